# revision 2
# baseline (speedup 1.0000x reference)
"""3-layer GCN forward (GCNConv x3 + log_softmax) on 8 Trainium2 cores.

Strategy (self-contained; shapes hardcoded for N=100000, Cin=Ch=128,
Cout=47, 8 cores):
  A_hat = D^-1/2 (A+I) D^-1/2 is fixed across layers, so per layer
      out = dinv_dst * segsum_dst( dinv_src * (H @ W) ) + b
  Host: permute nodes into 8 contiguous core blocks (degree-sorted within
  each block), build per-core padded gather grids: 98 groups of 128 output
  rows, each with d_g gather steps (shared loop structure across cores).
  Device (SPMD, one NEFF on cores 0-7):
    per layer: tiled GEMM + dinv_src row scale -> local Z block;
    AllGather Z across the 8 cores into a shared DRAM replica;
    aggregation: per group, d_g indirect-DMA row gathers (128 rows/instr)
    accumulated on the tensor engine via identity-matmul into PSUM;
    then dinv_dst scale + bias + relu (or log_softmax on the last layer).

z_full row space: node (core k, local r) lives at row k*12544 + r; rows
[12500, 12544) of each block are zero pads; ZROW (=12500) backs unused
grid slots.
"""
import numpy as np

NCORES = 8
N = 100000
NBLK = 12500
NPAD = 12544            # 98 * 128
NGRP = NPAD // 128      # 98
C = 128
COUT = 47
ZROW = NBLK             # a zero pad row (core 0 block)


def _preprocess(x, edge_index, W1, b1, W2, b2, W3, b3):
    x = np.asarray(x, np.float32)
    ei = np.asarray(edge_index)
    loop = np.arange(N, dtype=np.int64)
    src = np.concatenate([ei[0], loop]).astype(np.int64)
    dst = np.concatenate([ei[1], loop]).astype(np.int64)

    deg = np.bincount(dst, minlength=N).astype(np.float32)
    dinv = 1.0 / np.sqrt(np.maximum(deg, 1.0))

    # deal degree-ranked nodes round-robin across cores so all 8 cores'
    # group degree profiles align (minimizes cross-core max padding)
    rank = np.argsort(-deg, kind="stable")
    perm = np.empty(N, np.int64)
    for k in range(NCORES):
        perm[k * NBLK:(k + 1) * NBLK] = rank[k::NCORES]
    inv = np.empty(N, np.int64)
    inv[perm] = np.arange(N)

    srcp = inv[src]
    dstp = inv[dst]
    ksrc = srcp // NBLK
    srcg = ksrc * NPAD + (srcp - ksrc * NBLK)     # padded-global coords

    dinv_p = dinv[perm]

    ecore = dstp // NBLK
    rloc = dstp - ecore * NBLK
    order = np.lexsort((srcg, rloc, ecore))
    ecore, rloc, srcg_s = ecore[order], rloc[order], srcg[order]

    flat = ecore * NBLK + rloc                     # sorted
    cnt = np.bincount(flat, minlength=NCORES * NBLK)
    cnt_pad = np.zeros(NCORES * NPAD, np.int64)
    idx_all = (np.arange(NCORES * NBLK) // NBLK) * NPAD + \
        (np.arange(NCORES * NBLK) % NBLK)
    cnt_pad[idx_all] = cnt
    d_per = cnt_pad.reshape(NCORES, NGRP, 128).max(axis=2)
    d_g = np.maximum(d_per.max(axis=0), 1).astype(np.int64)
    col_off = np.concatenate([[0], np.cumsum(d_g)])
    n_steps = int(col_off[-1])

    tables = np.full((NCORES, 128, n_steps), ZROW, np.int32)
    starts = np.zeros(NCORES * NBLK + 1, np.int64)
    np.cumsum(cnt, out=starts[1:])
    pos = np.arange(len(order)) - starts[flat]
    grp = rloc // 128
    part = rloc % 128
    colidx = col_off[grp] + pos
    tables[ecore, part, colidx] = srcg_s.astype(np.int32)

    dinv_loc = np.zeros((NCORES, 128, NGRP), np.float32)
    dv = dinv_p.reshape(NCORES, NBLK)
    for k in range(NCORES):
        full = np.zeros(NPAD, np.float32)
        full[:NBLK] = dv[k]
        dinv_loc[k] = full.reshape(NGRP, 128).T

    xp = x[perm]
    xblk = np.zeros((NCORES, NPAD, C), np.float32)
    for k in range(NCORES):
        xblk[k, :NBLK] = xp[k * NBLK:(k + 1) * NBLK]

    Ws = [np.ascontiguousarray(W, np.float32) for W in (W1, W2, W3)]
    bb = [np.tile(np.asarray(b, np.float32)[None, :], (128, 1))
          for b in (b1, b2, b3)]

    in_maps = []
    for k in range(NCORES):
        in_maps.append({
            "xblk": np.ascontiguousarray(xblk[k]),
            "gidx": np.ascontiguousarray(tables[k]),
            "dinv": np.ascontiguousarray(dinv_loc[k]),
            "w1": Ws[0], "w2": Ws[1], "w3": Ws[2],
            "bb1": np.ascontiguousarray(bb[0]),
            "bb2": np.ascontiguousarray(bb[1]),
            "bb3": np.ascontiguousarray(bb[2]),
        })
    return in_maps, [int(v) for v in d_g], n_steps, perm


def _build(d_g, n_steps):
    from concourse import bacc, bass, mybir, tile
    from concourse.masks import make_identity
    f32 = mybir.dt.float32
    i32 = mybir.dt.int32
    couts = [C, C, COUT]

    nc = bacc.Bacc("TRN2", target_bir_lowering=False, debug=False,
                   num_devices=NCORES)
    xblk = nc.dram_tensor("xblk", [NPAD, C], f32, kind="ExternalInput")
    gidx = nc.dram_tensor("gidx", [128, n_steps], i32, kind="ExternalInput")
    dinv = nc.dram_tensor("dinv", [128, NGRP], f32, kind="ExternalInput")
    w_in = [nc.dram_tensor(f"w{l+1}", [C, couts[l]], f32,
                           kind="ExternalInput") for l in range(3)]
    bb_in = [nc.dram_tensor(f"bb{l+1}", [128, couts[l]], f32,
                            kind="ExternalInput") for l in range(3)]
    out_d = nc.dram_tensor("out", [NPAD, COUT], f32, kind="ExternalOutput")

    zsA = nc.dram_tensor("zsA", [NPAD, C], f32)          # layers 0,1
    zsB = nc.dram_tensor("zsB", [NPAD, COUT], f32)       # layer 2
    zf = [nc.dram_tensor(f"zf{l}", [NCORES * NPAD, couts[l]], f32,
                         addr_space="Shared") for l in range(3)]

    with tile.TileContext(nc) as tc:
        with tc.tile_pool(name="const", bufs=1) as cpool, \
             tc.tile_pool(name="hbuf", bufs=1) as hpool, \
             tc.tile_pool(name="gath", bufs=10) as gpool, \
             tc.tile_pool(name="work", bufs=4) as wpool, \
             tc.tile_pool(name="ps_t", bufs=2, space="PSUM") as ps_t, \
             tc.tile_pool(name="ps_z", bufs=2, space="PSUM") as ps_z, \
             tc.tile_pool(name="ps_g", bufs=2, space="PSUM") as ps_g:

            ident = cpool.tile([128, 128], f32)
            make_identity(nc, ident[:])
            idx_sb = cpool.tile([128, n_steps], i32)
            nc.sync.dma_start(out=idx_sb[:], in_=gidx[:])
            dinv_sb = cpool.tile([128, NGRP], f32)
            nc.sync.dma_start(out=dinv_sb[:], in_=dinv[:])
            w_sb, bb_sb = [], []
            for l in range(3):
                w = cpool.tile([128, couts[l]], f32, name=f"w_sb{l}")
                nc.sync.dma_start(out=w[:], in_=w_in[l][:])
                w_sb.append(w)
                b = cpool.tile([128, couts[l]], f32, name=f"bb_sb{l}")
                nc.sync.dma_start(out=b[:], in_=bb_in[l][:])
                bb_sb.append(b)

            H = hpool.tile([128, NGRP * C], f32)

            for lay in range(3):
                co = couts[lay]
                zs = zsA if lay < 2 else zsB
                for g in range(NGRP):
                    if lay == 0:
                        hin = wpool.tile([128, C], f32, name="hin")
                        nc.sync.dma_start(
                            out=hin[:], in_=xblk[g * 128:(g + 1) * 128, :])
                        hsrc = hin[:]
                    else:
                        hsrc = H[:, g * C:(g + 1) * C]
                    pst = ps_t.tile([128, 128], f32, name="pst")
                    nc.tensor.transpose(out=pst[:], in_=hsrc, identity=ident[:])
                    ht = wpool.tile([128, 128], f32, name="ht")
                    nc.vector.tensor_copy(out=ht[:], in_=pst[:])
                    psz = ps_z.tile([128, co], f32, name="psz")
                    nc.tensor.matmul(out=psz[:], lhsT=ht[:], rhs=w_sb[lay][:],
                                     start=True, stop=True)
                    zt = wpool.tile([128, C], f32, name="zt")
                    nc.vector.tensor_scalar_mul(out=zt[:, :co], in0=psz[:],
                                                scalar1=dinv_sb[:, g:g + 1])
                    nc.sync.dma_start(out=zs[g * 128:(g + 1) * 128, :],
                                      in_=zt[:, :co])

                nc.gpsimd.collective_compute(
                    "AllGather", mybir.AluOpType.bypass,
                    replica_groups=[list(range(NCORES))],
                    ins=[zs[:, :]], outs=[zf[lay][:, :]])

                s = 0
                for g in range(NGRP):
                    d = d_g[g]
                    nq = min(4, d)
                    psg = ps_g.tile([128, 4 * C], f32, name="psg")
                    nch = (d + 3) // 4
                    jj = 0
                    for ch in range(nch):
                        w = min(4, d - jj)
                        gs4 = gpool.tile([128, 4 * C], f32, name="gs")
                        for q in range(w):
                            nc.gpsimd.indirect_dma_start(
                                out=gs4[:, q * C:q * C + co], out_offset=None,
                                in_=zf[lay][:, :],
                                in_offset=bass.IndirectOffsetOnAxis(
                                    ap=idx_sb[:, s:s + 1], axis=0))
                            s += 1
                        nc.tensor.matmul(out=psg[:, :w * C], lhsT=ident[:],
                                         rhs=gs4[:, :w * C],
                                         start=(ch == 0), stop=(ch == nch - 1))
                        jj += w
                    tmp = wpool.tile([128, C], f32, name="tmp")
                    nc.vector.tensor_copy(out=tmp[:, :co], in_=psg[:, :co])
                    for q in range(1, nq):
                        nc.vector.tensor_add(out=tmp[:, :co], in0=tmp[:, :co],
                                             in1=psg[:, q * C:q * C + co])
                    nc.vector.tensor_scalar_mul(out=tmp[:, :co], in0=tmp[:, :co],
                                                scalar1=dinv_sb[:, g:g + 1])
                    nc.vector.tensor_add(out=tmp[:, :co], in0=tmp[:, :co],
                                         in1=bb_sb[lay][:])
                    if lay < 2:
                        nc.vector.tensor_scalar_max(
                            out=H[:, g * C:(g + 1) * C], in0=tmp[:, :co],
                            scalar1=0.0)
                    else:
                        mx = wpool.tile([128, 1], f32, name="mx")
                        nc.vector.tensor_reduce(
                            out=mx[:], in_=tmp[:, :co],
                            axis=mybir.AxisListType.X, op=mybir.AluOpType.max)
                        nmx = wpool.tile([128, 1], f32, name="nmx")
                        nc.vector.tensor_scalar_mul(out=nmx[:], in0=mx[:],
                                                    scalar1=-1.0)
                        ex = wpool.tile([128, C], f32, name="ex")
                        ssum = wpool.tile([128, 1], f32, name="ssum")
                        nc.scalar.activation(
                            out=ex[:, :co], in_=tmp[:, :co],
                            func=mybir.ActivationFunctionType.Exp,
                            bias=nmx[:], scale=1.0, accum_out=ssum[:])
                        lse = wpool.tile([128, 1], f32, name="lse")
                        nc.scalar.activation(
                            out=lse[:], in_=ssum[:],
                            func=mybir.ActivationFunctionType.Ln)
                        tot = wpool.tile([128, 1], f32, name="tot")
                        nc.vector.tensor_add(out=tot[:], in0=lse[:], in1=mx[:])
                        ot = wpool.tile([128, COUT], f32, name="ot")
                        nc.vector.tensor_scalar_sub(out=ot[:], in0=tmp[:, :co],
                                                    scalar1=tot[:])
                        nc.sync.dma_start(
                            out=out_d[g * 128:(g + 1) * 128, :], in_=ot[:])

    nc.compile()
    return nc


LAST_RES = None


def kernel(x, edge_index, W1, b1, W2, b2, W3, b3):
    import os
    from concourse.bass_utils import run_bass_kernel_spmd

    in_maps, d_g, n_steps, perm = _preprocess(
        x, edge_index, W1, b1, W2, b2, W3, b3)
    nc = _build(d_g, n_steps)
    kw = {}
    if os.environ.get("KERNEL_TRACE", "0") == "1":
        kw["trace"] = True
        if os.environ.get("KERNEL_TMPDIR"):
            kw["tmpdir"] = os.environ["KERNEL_TMPDIR"]
    res = run_bass_kernel_spmd(nc, in_maps, core_ids=list(range(NCORES)), **kw)
    global LAST_RES
    LAST_RES = res
    blocks = [res.results[k]["out"][:NBLK] for k in range(NCORES)]
    outp = np.concatenate(blocks, axis=0)
    out = np.empty((N, COUT), np.float32)
    out[perm] = outp
    return out



# revision 10
# speedup vs baseline: 1.9266x; 1.9266x over previous
"""3-layer GCN forward (GCNConv x3 + log_softmax) on 8 Trainium2 cores.

Strategy (self-contained; shapes hardcoded for N=100000, Cin=Ch=128,
Cout=47, 8 cores): A_hat = D^-1/2 (A+I) D^-1/2 fixed across layers, so
per layer out = dinv_dst * segsum_dst(dinv_src * (H @ W)) + b.

Host: permute nodes into 8 contiguous core blocks (degree-ranked
round-robin so all cores share one loop structure / NEFF). Per core,
edges are sorted into (dst-group g, source-quarter q) segments; each
segment is padded to 128-position tiles. The message gather uses
batched dma_gather instructions (int16 indices rebased per source
quarter of 25088 rows, 4 SWDGE queues in parallel), fetching bf16
feature rows of 256 B from the AllGathered Z replica.

Aggregation per tile of 128 messages: build a selection matrix
SEL[p, r] = (dst_id[p] == r) with one DVE is_equal op (4 tiles per op
via a stride-0 broadcast), then matmul(lhsT=SEL, rhs=messages) into
the group's PSUM accumulator. Bias is a rank-1 matmul
(binv x b, binv = 1/dinv so the later dinv_dst scale restores b).
Post per group: one fused scale+relu, PE transpose, next-layer GEMM,
dinv_src scale, zs write. AllGather (bf16) exchanges Z between layers.
Final layer: scale + log_softmax.
"""
import numpy as np
import ml_dtypes

NCORES = 8
N = 100000
NBLK = 12500
NPAD = 12544            # 98 * 128
NGRP = NPAD // 128      # 98
C = 128
COUT = 47
QROWS = 25088           # rows per source quarter (2 core blocks)
NQ = 4
GBLK = 4                # groups per gather-chunk block


def _preprocess(x, edge_index, W1, b1, W2, b2, W3, b3):
    x = np.asarray(x, np.float32)
    ei = np.asarray(edge_index)
    loop = np.arange(N, dtype=np.int64)
    src = np.concatenate([ei[0], loop]).astype(np.int64)
    dst = np.concatenate([ei[1], loop]).astype(np.int64)

    deg = np.bincount(dst, minlength=N).astype(np.float32)
    dinv = 1.0 / np.sqrt(np.maximum(deg, 1.0))

    rank = np.argsort(-deg, kind="stable")
    perm = np.empty(N, np.int64)
    for k in range(NCORES):
        perm[k * NBLK:(k + 1) * NBLK] = rank[k::NCORES]
    inv = np.empty(N, np.int64)
    inv[perm] = np.arange(N)

    srcp = inv[src]
    dstp = inv[dst]
    ksrc = srcp // NBLK
    srcg = ksrc * NPAD + (srcp - ksrc * NBLK)     # padded-global coords
    dinv_p = dinv[perm]

    ecore = dstp // NBLK
    rloc = dstp - ecore * NBLK
    grp = rloc // 128
    gpart = rloc % 128
    qq = srcg // QROWS

    # counts per (core, group, quarter) -> shared tile structure
    key = (ecore * NGRP + grp) * NQ + qq
    cnt = np.bincount(key, minlength=NCORES * NGRP * NQ) \
        .reshape(NCORES, NGRP, NQ)
    tiles_gq = np.maximum(
        np.ceil(cnt / 128).astype(np.int64).max(axis=0), 1)   # [NGRP, NQ]

    # quarter-major stream layout: for q: for g: tiles_gq[g, q] tiles
    tile_base_q = np.zeros(NQ + 1, np.int64)
    tile_base_q[1:] = np.cumsum(tiles_gq.sum(axis=0))
    # tile index of (g, q): tile_base_q[q] + cumsum over g
    tile_off_gq = np.zeros((NGRP, NQ), np.int64)
    for q in range(NQ):
        tile_off_gq[1:, q] = np.cumsum(tiles_gq[:-1, q])
    TT = int(tile_base_q[-1])          # total tiles per core per layer

    # build per-core idx16 stream + ids stream
    # sort by (core, group, quarter) to match the segment/`within` layout
    order = np.lexsort((srcg, rloc, qq, grp, ecore))
    e_core = ecore[order]
    e_grp = grp[order]
    e_gpart = gpart[order]
    e_qq = qq[order]
    e_src = srcg[order]

    idx16 = np.full((NCORES, TT * 128), 12500, np.int16)   # pad -> quarter pad row
    idsarr = np.full((NCORES, TT * 128), -1.0, ml_dtypes.bfloat16)

    # stream position for each edge: within (core, g, q) segment
    ckey = (e_core * NGRP + e_grp) * NQ + e_qq
    starts = np.zeros(NCORES * NGRP * NQ + 1, np.int64)
    np.cumsum(cnt.reshape(-1), out=starts[1:])
    within = np.arange(len(order)) - starts[ckey]
    gtile = (tile_base_q[e_qq] + tile_off_gq[e_grp, e_qq]) * 128 + within
    idx16[e_core, gtile] = (e_src - e_qq * QROWS).astype(np.int16)
    idsarr[e_core, gtile] = e_gpart.astype(np.float32)

    # idx16 SBUF layout: stream pos i -> partition i%16 (replicated x8),
    # free slot i//16
    idxw = TT * 128 // 16
    idx_sb = np.zeros((NCORES, 128, idxw), np.int16)
    w16 = idx16.reshape(NCORES, idxw, 16).transpose(0, 2, 1)  # [8,16,idxw]
    idx_sb[:] = np.tile(w16, (1, 8, 1))
    # ids SBUF layout: [128, TT]: tile t, partition p -> id
    ids_sb = np.ascontiguousarray(
        idsarr.reshape(NCORES, TT, 128).transpose(0, 2, 1))

    # chunk blocks: groups [GBLK*b, GBLK*(b+1)) per quarter
    nblocks = (NGRP + GBLK - 1) // GBLK
    chunks = []   # [block][q] = (idx_slot_start, n_idx, tile_list)
    for b in range(nblocks):
        row = []
        g0, g1 = b * GBLK, min((b + 1) * GBLK, NGRP)
        for q in range(NQ):
            t0 = tile_base_q[q] + tile_off_gq[g0, q]
            ntile = int(tiles_gq[g0:g1, q].sum())
            row.append((int(t0), ntile))
        chunks.append(row)

    dinv_loc = np.zeros((NCORES, 128, NGRP), np.float32)
    binv_row = np.zeros((NCORES, 1, NPAD), np.float32)
    dv = dinv_p.reshape(NCORES, NBLK)
    for k in range(NCORES):
        full = np.zeros(NPAD, np.float32)
        full[:NBLK] = dv[k]
        dinv_loc[k] = full.reshape(NGRP, 128).T
        with np.errstate(divide="ignore"):
            bi = np.where(full > 0, 1.0 / full, 0.0)
        binv_row[k, 0] = bi

    xp = x[perm]
    xblkT = np.zeros((NCORES, C, NPAD), np.float32)
    for k in range(NCORES):
        xblkT[k, :, :NBLK] = xp[k * NBLK:(k + 1) * NBLK].T

    Ws = [np.ascontiguousarray(W, np.float32) for W in (W1, W2, W3)]
    brows = [np.asarray(b, ml_dtypes.bfloat16).reshape(1, -1)
             for b in (b1, b2, b3)]
    iota4 = np.tile(np.arange(128, dtype=np.float32)[None, :],
                    (128, 4)).astype(ml_dtypes.bfloat16)

    in_maps = []
    for k in range(NCORES):
        in_maps.append({
            "xblkT": np.ascontiguousarray(xblkT[k]),
            "gidx": np.ascontiguousarray(idx_sb[k]),
            "gids": np.ascontiguousarray(ids_sb[k].astype(ml_dtypes.bfloat16)),
            "dinv": np.ascontiguousarray(dinv_loc[k]),
            "binv": np.ascontiguousarray(
                binv_row[k].astype(ml_dtypes.bfloat16)),
            "iota4": iota4,
            "w1": Ws[0], "w2": Ws[1], "w3": Ws[2],
            "br1": brows[0], "br2": brows[1], "br3": brows[2],
        })
    meta = {
        "TT": TT, "idxw": idxw,
        "tiles_gq": tiles_gq.tolist(),
        "tile_base_q": tile_base_q.tolist(),
        "tile_off_gq": tile_off_gq.tolist(),
        "chunks": chunks,
    }
    return in_maps, meta, perm


def _build(meta):
    from concourse import bacc, bass, mybir, tile
    from concourse.masks import make_identity
    f32 = mybir.dt.float32
    bf16 = mybir.dt.bfloat16
    i16 = mybir.dt.int16

    TT = meta["TT"]
    idxw = meta["idxw"]
    tiles_gq = meta["tiles_gq"]
    tile_base_q = meta["tile_base_q"]
    tile_off_gq = meta["tile_off_gq"]
    chunks = meta["chunks"]
    nblocks = len(chunks)
    maxtile = max(c[1] for row in chunks for c in [row[q] for q in range(NQ)])

    nc = bacc.Bacc("TRN2", target_bir_lowering=False, debug=False,
                   num_devices=NCORES, num_swdge_queues=4)
    xTd = nc.dram_tensor("xblkT", [C, NPAD], f32, kind="ExternalInput")
    gidx = nc.dram_tensor("gidx", [128, idxw], i16, kind="ExternalInput")
    gids = nc.dram_tensor("gids", [128, TT], bf16, kind="ExternalInput")
    dinv = nc.dram_tensor("dinv", [128, NGRP], f32, kind="ExternalInput")
    binv = nc.dram_tensor("binv", [1, NPAD], bf16, kind="ExternalInput")
    iota_in = nc.dram_tensor("iota4", [128, 4 * 128], bf16,
                             kind="ExternalInput")
    w_in = [nc.dram_tensor(f"w{l+1}", [C, co], f32, kind="ExternalInput")
            for l, co in enumerate([C, C, COUT])]
    br_in = [nc.dram_tensor(f"br{l+1}", [1, co], bf16, kind="ExternalInput")
             for l, co in enumerate([C, C, COUT])]
    out_d = nc.dram_tensor("out", [NPAD, COUT], f32, kind="ExternalOutput")

    zs = [nc.dram_tensor(f"zs{l}", [NPAD, C], bf16) for l in range(3)]
    zf = [nc.dram_tensor(f"zf{l}", [NCORES * NPAD, C], bf16,
                         addr_space="Shared") for l in range(3)]

    with tile.TileContext(nc) as tc:
        with tc.tile_pool(name="const", bufs=1) as cpool, \
             tc.tile_pool(name="g0", bufs=2) as gp0, \
             tc.tile_pool(name="g1", bufs=2) as gp1, \
             tc.tile_pool(name="g2", bufs=2) as gp2, \
             tc.tile_pool(name="g3", bufs=2) as gp3, \
             tc.tile_pool(name="sel", bufs=4) as selpool, \
             tc.tile_pool(name="work", bufs=4) as wpool, \
             tc.tile_pool(name="ps_g", bufs=2, space="PSUM") as ps_g, \
             tc.tile_pool(name="ps_t", bufs=2, space="PSUM") as ps_t, \
             tc.tile_pool(name="ps_z", bufs=2, space="PSUM") as ps_z:
            gpools = [gp0, gp1, gp2, gp3]

            ident = cpool.tile([128, 128], f32)
            make_identity(nc, ident[:])
            iota4 = cpool.tile([128, 4, 128], bf16)
            nc.sync.dma_start(out=iota4[:], in_=iota_in[:])
            idx_sb = cpool.tile([128, idxw], i16)
            nc.sync.dma_start(out=idx_sb[:], in_=gidx[:])
            ids_sb = cpool.tile([128, TT], bf16)
            nc.sync.dma_start(out=ids_sb[:], in_=gids[:])
            dinv_sb = cpool.tile([128, NGRP], f32)
            nc.sync.dma_start(out=dinv_sb[:], in_=dinv[:])
            binv_sb = cpool.tile([1, NPAD], bf16)
            nc.sync.dma_start(out=binv_sb[:], in_=binv[:])
            w_sb, br_sb = [], []
            for l, co in enumerate([C, C, COUT]):
                w = cpool.tile([128, co], f32, name=f"w_sb{l}")
                nc.sync.dma_start(out=w[:], in_=w_in[l][:])
                w_sb.append(w)
                b = cpool.tile([1, co], bf16, name=f"br_sb{l}")
                nc.sync.dma_start(out=b[:], in_=br_in[l][:])
                br_sb.append(b)

            # ---- layer-1 GEMM: zs0 = dinv * (x @ W1), cast bf16 ----
            for g in range(NGRP):
                xt = wpool.tile([128, 128], f32, name="xt")
                nc.sync.dma_start(out=xt[:],
                                  in_=xTd[:, g * 128:(g + 1) * 128])
                psz = ps_z.tile([128, C], f32, name="psz")
                nc.tensor.matmul(out=psz[:], lhsT=xt[:], rhs=w_sb[0][:],
                                 start=True, stop=True)
                zt = wpool.tile([128, C], bf16, name="zt")
                nc.vector.tensor_scalar_mul(out=zt[:], in0=psz[:],
                                            scalar1=dinv_sb[:, g:g + 1])
                nc.sync.dma_start(out=zs[0][g * 128:(g + 1) * 128, :],
                                  in_=zt[:])

            nc.gpsimd.collective_compute(
                "AllGather", mybir.AluOpType.bypass,
                replica_groups=[list(range(NCORES))],
                ins=[zs[0][:, :]], outs=[zf[0][:, :]])

            # ---- per layer: stream-gather aggregation (+ GEMM fusion) ----
            for lay in range(3):
                for b in range(nblocks):
                    bufs = []
                    for q in range(NQ):
                        t0, ntile = chunks[b][q]
                        buf = gpools[q].tile([128, maxtile, C], bf16,
                                             name=f"gb{q}")
                        # <=8 tiles (1024 idxs) per instruction: larger
                        # descriptor batches can exceed the SWDGE ring
                        for s0 in range(0, ntile, 8):
                            ns = min(8, ntile - s0)
                            nc.gpsimd.dma_gather(
                                buf[:, s0:s0 + ns, :],
                                zf[lay][q * QROWS:(q + 1) * QROWS, :],
                                idx_sb[:, (t0 + s0) * 8:(t0 + s0 + ns) * 8],
                                ns * 128, ns * 128, C, queue_num=q)
                        bufs.append((buf, t0))
                    g0 = b * GBLK
                    for g in range(g0, min(g0 + GBLK, NGRP)):
                        psg = ps_g.tile([128, C], f32, name="psg")
                        first = True
                        for q in range(NQ):
                            buf, t0 = bufs[q]
                            tg0 = tile_base_q[q] + tile_off_gq[g][q]
                            nt = tiles_gq[g][q]
                            coff = tg0 - (tile_base_q[q] + tile_off_gq[g0][q])
                            for j0 in range(0, nt, 4):
                                w4 = min(4, nt - j0)
                                sel = selpool.tile([128, 4, 128], bf16,
                                                   name="sel")
                                nc.vector.tensor_tensor(
                                    out=sel[:, :w4, :],
                                    in0=iota4[:, :w4, :],
                                    in1=ids_sb[:, tg0 + j0:tg0 + j0 + w4]
                                        .to_broadcast([128, w4, 128]),
                                    op=mybir.AluOpType.is_equal)
                                for j in range(w4):
                                    nc.tensor.matmul(
                                        out=psg[:],
                                        lhsT=sel[:, j, :],
                                        rhs=buf[:, coff + j0 + j, :],
                                        start=first, stop=False)
                                    first = False
                        # rank-1 bias: psg += binv_g (x) b_row
                        co = C if lay < 2 else COUT
                        nc.tensor.matmul(
                            out=psg[:, :co],
                            lhsT=binv_sb[:, g * 128:(g + 1) * 128],
                            rhs=br_sb[lay][:], start=False, stop=True)
                        if lay < 2:
                            h = wpool.tile([128, 128], f32, name="h")
                            nc.vector.tensor_scalar(
                                out=h[:], in0=psg[:],
                                scalar1=dinv_sb[:, g:g + 1], scalar2=0.0,
                                op0=mybir.AluOpType.mult,
                                op1=mybir.AluOpType.max)
                            pst = ps_t.tile([128, 128], f32, name="pst")
                            nc.tensor.transpose(out=pst[:], in_=h[:],
                                                identity=ident[:])
                            ht = wpool.tile([128, 128], f32, name="ht")
                            nc.vector.tensor_copy(out=ht[:], in_=pst[:])
                            co2 = C if lay == 0 else COUT
                            psz = ps_z.tile([128, C], f32, name="psz2")
                            nc.tensor.matmul(out=psz[:, :co2], lhsT=ht[:],
                                             rhs=w_sb[lay + 1][:],
                                             start=True, stop=True)
                            zt = wpool.tile([128, C], bf16, name="zt2")
                            nc.vector.tensor_scalar_mul(
                                out=zt[:, :co2], in0=psz[:, :co2],
                                scalar1=dinv_sb[:, g:g + 1])
                            nc.sync.dma_start(
                                out=zs[lay + 1][g * 128:(g + 1) * 128, :co2],
                                in_=zt[:, :co2])
                        else:
                            tmp = wpool.tile([128, COUT], f32, name="tmp")
                            nc.vector.tensor_scalar_mul(
                                out=tmp[:], in0=psg[:, :COUT],
                                scalar1=dinv_sb[:, g:g + 1])
                            mx = wpool.tile([128, 1], f32, name="mx")
                            nc.vector.tensor_reduce(
                                out=mx[:], in_=tmp[:],
                                axis=mybir.AxisListType.X,
                                op=mybir.AluOpType.max)
                            nmx = wpool.tile([128, 1], f32, name="nmx")
                            nc.vector.tensor_scalar_mul(
                                out=nmx[:], in0=mx[:], scalar1=-1.0)
                            ex = wpool.tile([128, COUT], f32, name="ex")
                            ssum = wpool.tile([128, 1], f32, name="ssum")
                            nc.scalar.activation(
                                out=ex[:], in_=tmp[:],
                                func=mybir.ActivationFunctionType.Exp,
                                bias=nmx[:], scale=1.0, accum_out=ssum[:])
                            lse = wpool.tile([128, 1], f32, name="lse")
                            nc.scalar.activation(
                                out=lse[:], in_=ssum[:],
                                func=mybir.ActivationFunctionType.Ln)
                            tot = wpool.tile([128, 1], f32, name="tot")
                            nc.vector.tensor_add(out=tot[:], in0=lse[:],
                                                 in1=mx[:])
                            ot = wpool.tile([128, COUT], f32, name="ot")
                            nc.vector.tensor_scalar_sub(
                                out=ot[:], in0=tmp[:], scalar1=tot[:])
                            nc.sync.dma_start(
                                out=out_d[g * 128:(g + 1) * 128, :],
                                in_=ot[:])
                if lay < 2:
                    nc.gpsimd.collective_compute(
                        "AllGather", mybir.AluOpType.bypass,
                        replica_groups=[list(range(NCORES))],
                        ins=[zs[lay + 1][:, :]], outs=[zf[lay + 1][:, :]])

    nc.compile()
    return nc


LAST_RES = None


def kernel(x, edge_index, W1, b1, W2, b2, W3, b3):
    import os
    from concourse.bass_utils import run_bass_kernel_spmd

    in_maps, meta, perm = _preprocess(
        x, edge_index, W1, b1, W2, b2, W3, b3)
    nc = _build(meta)
    kw = {}
    if os.environ.get("KERNEL_TRACE", "0") == "1":
        kw["trace"] = True
        if os.environ.get("KERNEL_TMPDIR"):
            kw["tmpdir"] = os.environ["KERNEL_TMPDIR"]
    res = run_bass_kernel_spmd(nc, in_maps, core_ids=list(range(NCORES)), **kw)
    global LAST_RES
    LAST_RES = res
    blocks = [res.results[k]["out"][:NBLK] for k in range(NCORES)]
    outp = np.concatenate(blocks, axis=0)
    out = np.empty((N, COUT), np.float32)
    out[perm] = outp
    return out


# revision 27
# speedup vs baseline: 2.0084x; 1.0425x over previous
"""3-layer GCN forward (GCNConv x3 + log_softmax) on 8 Trainium2 cores.

Strategy (self-contained; shapes hardcoded for N=100000, Cin=Ch=128,
Cout=47, 8 cores): A_hat = D^-1/2 (A+I) D^-1/2 fixed across layers, so
per layer out = dinv_dst * segsum_dst(dinv_src * (H @ W)) + b.

Host: permute nodes into 8 contiguous core blocks (degree-ranked
round-robin so all cores share one loop structure / NEFF). Per core,
edges are sorted into (dst-group g, source-quarter q) segments; each
segment is padded to 128-position tiles. The message gather uses
batched dma_gather instructions (int16 indices rebased per source
quarter of 25088 rows, 4 SWDGE queues in parallel), fetching bf16
feature rows of 256 B from the AllGathered Z replica.

Aggregation per tile of 128 messages: build a selection matrix
SEL[p, r] = (dst_id[p] == r) with one DVE is_equal op (4 tiles per op
via a stride-0 broadcast), then matmul(lhsT=SEL, rhs=messages) into
the group's PSUM accumulator. Bias is a rank-1 matmul
(binv x b, binv = 1/dinv so the later dinv_dst scale restores b).
Post per group: one fused scale+relu, PE transpose, next-layer GEMM,
dinv_src scale, zs write. AllGather (bf16) exchanges Z between layers.
Final layer: scale + log_softmax.
"""
import numpy as np
import ml_dtypes

NCORES = 8
N = 100000
NBLK = 12500
NPAD = 12544            # 98 * 128
NGRP = NPAD // 128      # 98
C = 128
COUT = 47
QROWS = 25088           # rows per source quarter (2 core blocks)
NQ = 4
GBLK = 4                # groups per gather-chunk block


def _preprocess(x, edge_index, W1, b1, W2, b2, W3, b3):
    x = np.asarray(x, np.float32)
    ei = np.asarray(edge_index)
    loop = np.arange(N, dtype=np.int64)
    src = np.concatenate([ei[0], loop]).astype(np.int64)
    dst = np.concatenate([ei[1], loop]).astype(np.int64)

    deg = np.bincount(dst, minlength=N).astype(np.float32)
    dinv = 1.0 / np.sqrt(np.maximum(deg, 1.0))

    rank = np.argsort(-deg, kind="stable")
    perm = np.empty(N, np.int64)
    for k in range(NCORES):
        perm[k * NBLK:(k + 1) * NBLK] = rank[k::NCORES]
    inv = np.empty(N, np.int64)
    inv[perm] = np.arange(N)

    srcp = inv[src]
    dstp = inv[dst]
    ksrc = srcp // NBLK
    srcg = ksrc * NPAD + (srcp - ksrc * NBLK)     # padded-global coords
    dinv_p = dinv[perm]

    ecore = dstp // NBLK
    rloc = dstp - ecore * NBLK
    grp = rloc // 128
    gpart = rloc % 128
    qq = srcg // QROWS

    # counts per (core, group, quarter) -> shared tile structure
    key = (ecore * NGRP + grp) * NQ + qq
    cnt = np.bincount(key, minlength=NCORES * NGRP * NQ) \
        .reshape(NCORES, NGRP, NQ)
    tiles_gq = np.maximum(
        np.ceil(cnt / 128).astype(np.int64).max(axis=0), 1)   # [NGRP, NQ]

    # quarter-major stream layout: for q: for g: tiles_gq[g, q] tiles
    tile_base_q = np.zeros(NQ + 1, np.int64)
    tile_base_q[1:] = np.cumsum(tiles_gq.sum(axis=0))
    # tile index of (g, q): tile_base_q[q] + cumsum over g
    tile_off_gq = np.zeros((NGRP, NQ), np.int64)
    for q in range(NQ):
        tile_off_gq[1:, q] = np.cumsum(tiles_gq[:-1, q])
    TT = int(tile_base_q[-1])          # total tiles per core per layer

    # build per-core idx16 stream + ids stream
    # sort by (core, group, quarter) to match the segment/`within` layout
    order = np.lexsort((srcg, rloc, qq, grp, ecore))
    e_core = ecore[order]
    e_grp = grp[order]
    e_gpart = gpart[order]
    e_qq = qq[order]
    e_src = srcg[order]

    idx16 = np.full((NCORES, TT * 128), 12500, np.int16)   # pad -> quarter pad row
    idsarr = np.full((NCORES, TT * 128), -1.0, ml_dtypes.bfloat16)

    # stream position for each edge: within (core, g, q) segment
    ckey = (e_core * NGRP + e_grp) * NQ + e_qq
    starts = np.zeros(NCORES * NGRP * NQ + 1, np.int64)
    np.cumsum(cnt.reshape(-1), out=starts[1:])
    within = np.arange(len(order)) - starts[ckey]
    gtile = (tile_base_q[e_qq] + tile_off_gq[e_grp, e_qq]) * 128 + within
    idx16[e_core, gtile] = (e_src - e_qq * QROWS).astype(np.int16)
    idsarr[e_core, gtile] = e_gpart.astype(np.float32)

    # idx16 SBUF layout: stream pos i -> partition i%16 (replicated x8),
    # free slot i//16
    idxw = TT * 128 // 16
    idx_sb = np.zeros((NCORES, 128, idxw), np.int16)
    w16 = idx16.reshape(NCORES, idxw, 16).transpose(0, 2, 1)  # [8,16,idxw]
    idx_sb[:] = np.tile(w16, (1, 8, 1))
    # ids SBUF layout: [128, TT] in GROUP-major tile order (g, then q, t)
    # so each group's SEL builds batch over contiguous columns
    gm_off = np.zeros((NGRP, NQ), np.int64)
    gm_off[:, 1:] = np.cumsum(tiles_gq[:, :-1], axis=1)
    tiles_tot = tiles_gq.sum(axis=1)
    gm_base = np.zeros(NGRP, np.int64)
    gm_base[1:] = np.cumsum(tiles_tot[:-1])
    perm_t = np.zeros(TT, np.int64)
    for g in range(NGRP):
        for q in range(NQ):
            for t in range(tiles_gq[g, q]):
                perm_t[gm_base[g] + gm_off[g, q] + t] = \
                    tile_base_q[q] + tile_off_gq[g, q] + t
    ids_q = idsarr.reshape(NCORES, TT, 128)
    ids_sb = np.ascontiguousarray(
        ids_q[:, perm_t, :].transpose(0, 2, 1))

    # chunk blocks: groups [GBLK*b, GBLK*(b+1)) per quarter
    nblocks = (NGRP + GBLK - 1) // GBLK
    chunks = []   # [block][q] = (idx_slot_start, n_idx, tile_list)
    for b in range(nblocks):
        row = []
        g0, g1 = b * GBLK, min((b + 1) * GBLK, NGRP)
        for q in range(NQ):
            t0 = tile_base_q[q] + tile_off_gq[g0, q]
            ntile = int(tiles_gq[g0:g1, q].sum())
            row.append((int(t0), ntile))
        chunks.append(row)

    dinv_loc = np.zeros((NCORES, 128, NGRP), np.float32)
    binv_row = np.zeros((NCORES, 1, NPAD), np.float32)
    dv = dinv_p.reshape(NCORES, NBLK)
    for k in range(NCORES):
        full = np.zeros(NPAD, np.float32)
        full[:NBLK] = dv[k]
        dinv_loc[k] = full.reshape(NGRP, 128).T
        with np.errstate(divide="ignore"):
            bi = np.where(full > 0, 1.0 / full, 0.0)
        binv_row[k, 0] = bi

    xp = x[perm]
    xblkT = np.zeros((NCORES, C, NPAD), np.float32)
    for k in range(NCORES):
        xblkT[k, :, :NBLK] = xp[k * NBLK:(k + 1) * NBLK].T

    Ws = [np.ascontiguousarray(W, np.float32) for W in (W1, W2, W3)]
    brows = [np.asarray(b, ml_dtypes.bfloat16).reshape(1, -1)
             for b in (b1, b2, b3)]
    has_bias = any(np.any(np.asarray(b) != 0) for b in (b1, b2, b3))
    iota4 = np.tile(np.arange(128, dtype=np.float32)[None, :],
                    (128, 16)).astype(ml_dtypes.bfloat16)

    in_maps = []
    for k in range(NCORES):
        m = {
            "xblkT": np.ascontiguousarray(xblkT[k]),
            "gidx": np.ascontiguousarray(idx_sb[k]),
            "gids": np.ascontiguousarray(ids_sb[k].astype(ml_dtypes.bfloat16)),
            "dinv": np.ascontiguousarray(dinv_loc[k]),
            "iota4": iota4,
            "w1": Ws[0], "w2": Ws[1], "w3": Ws[2],
        }
        if has_bias:
            m["binv"] = np.ascontiguousarray(
                binv_row[k].astype(ml_dtypes.bfloat16))
            m["br1"], m["br2"], m["br3"] = brows
        in_maps.append(m)
    meta = {
        "TT": TT, "idxw": idxw,
        "tiles_gq": tiles_gq.tolist(),
        "tile_base_q": tile_base_q.tolist(),
        "tile_off_gq": tile_off_gq.tolist(),
        "gm_base": gm_base.tolist(),
        "chunks": chunks,
        "has_bias": bool(has_bias),
    }
    return in_maps, meta, perm


def _build(meta):
    from concourse import bacc, bass, mybir, tile
    from concourse.masks import make_identity
    f32 = mybir.dt.float32
    bf16 = mybir.dt.bfloat16
    i16 = mybir.dt.int16

    TT = meta["TT"]
    idxw = meta["idxw"]
    tiles_gq = meta["tiles_gq"]
    tile_base_q = meta["tile_base_q"]
    tile_off_gq = meta["tile_off_gq"]
    gm_base = meta["gm_base"]
    chunks = meta["chunks"]
    nblocks = len(chunks)
    maxtile = max(c[1] for row in chunks for c in [row[q] for q in range(NQ)])

    nc = bacc.Bacc("TRN2", target_bir_lowering=False, debug=False,
                   num_devices=NCORES, num_swdge_queues=4)
    xTd = nc.dram_tensor("xblkT", [C, NPAD], f32, kind="ExternalInput")
    gidx = nc.dram_tensor("gidx", [128, idxw], i16, kind="ExternalInput")
    gids = nc.dram_tensor("gids", [128, TT], bf16, kind="ExternalInput")
    dinv = nc.dram_tensor("dinv", [128, NGRP], f32, kind="ExternalInput")
    has_bias = meta["has_bias"]
    iota_in = nc.dram_tensor("iota4", [128, 16 * 128], bf16,
                             kind="ExternalInput")
    w_in = [nc.dram_tensor(f"w{l+1}", [C, co], f32, kind="ExternalInput")
            for l, co in enumerate([C, C, COUT])]
    if has_bias:
        binv = nc.dram_tensor("binv", [1, NPAD], bf16, kind="ExternalInput")
        br_in = [nc.dram_tensor(f"br{l+1}", [1, co], bf16,
                                kind="ExternalInput")
                 for l, co in enumerate([C, C, COUT])]
    out_d = nc.dram_tensor("out", [NPAD, COUT], f32, kind="ExternalOutput")

    zs = [nc.dram_tensor(f"zs{l}", [NPAD, C], bf16) for l in range(3)]
    zf = [nc.dram_tensor(f"zf{l}", [NCORES * NPAD, C], bf16,
                         addr_space="Shared") for l in range(3)]

    with tile.TileContext(nc) as tc:
        with tc.tile_pool(name="const", bufs=1) as cpool, \
             tc.tile_pool(name="g0", bufs=2) as gp0, \
             tc.tile_pool(name="g1", bufs=2) as gp1, \
             tc.tile_pool(name="g2", bufs=2) as gp2, \
             tc.tile_pool(name="g3", bufs=2) as gp3, \
             tc.tile_pool(name="sel", bufs=4) as selpool, \
             tc.tile_pool(name="work", bufs=4) as wpool, \
             tc.tile_pool(name="ps_g", bufs=2, space="PSUM") as ps_g, \
             tc.tile_pool(name="ps_t", bufs=2, space="PSUM") as ps_t, \
             tc.tile_pool(name="ps_z", bufs=2, space="PSUM") as ps_z:
            gpools = [gp0, gp1, gp2, gp3]

            ident = cpool.tile([128, 128], f32)
            make_identity(nc, ident[:])
            iota4 = cpool.tile([128, 16, 128], bf16)
            nc.sync.dma_start(out=iota4[:], in_=iota_in[:])
            smbuf = cpool.tile([128, NGRP, COUT], f32)
            smx = cpool.tile([128, NGRP], f32)
            sls = cpool.tile([128, NGRP], f32)
            idx_sb = cpool.tile([128, idxw], i16)
            nc.sync.dma_start(out=idx_sb[:], in_=gidx[:])
            ids_sb = cpool.tile([128, TT], bf16)
            nc.sync.dma_start(out=ids_sb[:], in_=gids[:])
            dinv_sb = cpool.tile([128, NGRP], f32)
            nc.sync.dma_start(out=dinv_sb[:], in_=dinv[:])
            w_sb, br_sb = [], []
            for l, co in enumerate([C, C, COUT]):
                w = cpool.tile([128, co], f32, name=f"w_sb{l}")
                nc.sync.dma_start(out=w[:], in_=w_in[l][:])
                w_sb.append(w)
            if has_bias:
                binv_sb = cpool.tile([1, NPAD], bf16)
                nc.sync.dma_start(out=binv_sb[:], in_=binv[:])
                for l, co in enumerate([C, C, COUT]):
                    bt = cpool.tile([1, co], bf16, name=f"br_sb{l}")
                    nc.sync.dma_start(out=bt[:], in_=br_in[l][:])
                    br_sb.append(bt)

            # ---- layer-1 GEMM: zs0 = dinv * (x @ W1), cast bf16 ----
            for g in range(NGRP):
                xt = wpool.tile([128, 128], f32, name="xt")
                nc.sync.dma_start(out=xt[:],
                                  in_=xTd[:, g * 128:(g + 1) * 128])
                psz = ps_z.tile([128, C], f32, name="psz")
                nc.tensor.matmul(out=psz[:], lhsT=xt[:], rhs=w_sb[0][:],
                                 start=True, stop=True)
                zt = wpool.tile([128, C], bf16, name="zt")
                nc.vector.tensor_scalar_mul(out=zt[:], in0=psz[:],
                                            scalar1=dinv_sb[:, g:g + 1])
                nc.sync.dma_start(out=zs[0][g * 128:(g + 1) * 128, :],
                                  in_=zt[:])

            nc.gpsimd.collective_compute(
                "AllGather", mybir.AluOpType.bypass,
                replica_groups=[list(range(NCORES))],
                ins=[zs[0][:, :]], outs=[zf[0][:, :]])

            # ---- per layer: stream-gather aggregation (+ GEMM fusion) ----
            for lay in range(3):
                for b in range(nblocks):
                    bufs = []
                    for q in range(NQ):
                        t0, ntile = chunks[b][q]
                        buf = gpools[q].tile([128, maxtile, C], bf16,
                                             name=f"gb{q}")
                        # <=8 tiles (1024 idxs) per instruction: larger
                        # descriptor batches can exceed the SWDGE ring
                        for s0 in range(0, ntile, 8):
                            ns = min(8, ntile - s0)
                            nc.gpsimd.dma_gather(
                                buf[:, s0:s0 + ns, :],
                                zf[lay][q * QROWS:(q + 1) * QROWS, :],
                                idx_sb[:, (t0 + s0) * 8:(t0 + s0 + ns) * 8],
                                ns * 128, ns * 128, C, queue_num=q)
                        bufs.append((buf, t0))
                    g0 = b * GBLK
                    for g in range(g0, min(g0 + GBLK, NGRP)):
                        # SEL mega-batches over the group's contiguous
                        # group-major ids columns (16 tiles per DVE op)
                        gmb = gm_base[g]
                        ntg = sum(tiles_gq[g])
                        sels = []
                        for s0 in range(0, ntg, 16):
                            w16 = min(16, ntg - s0)
                            selt = selpool.tile([128, 16, 128], bf16,
                                                name="sel")
                            nc.vector.tensor_tensor(
                                out=selt[:, :w16, :],
                                in0=iota4[:, :w16, :],
                                in1=ids_sb[:, gmb + s0:gmb + s0 + w16]
                                    .to_broadcast([128, w16, 128]),
                                op=mybir.AluOpType.is_equal)
                            sels.append(selt)
                        psg = ps_g.tile([128, C], f32, name="psg")
                        jg = 0
                        for q in range(NQ):
                            buf, t0 = bufs[q]
                            tg0 = tile_base_q[q] + tile_off_gq[g][q]
                            nt = tiles_gq[g][q]
                            coff = tg0 - (tile_base_q[q] + tile_off_gq[g0][q])
                            for j in range(nt):
                                nc.tensor.matmul(
                                    out=psg[:],
                                    lhsT=sels[jg // 16][:, jg % 16, :],
                                    rhs=buf[:, coff + j, :],
                                    start=(jg == 0),
                                    stop=(not has_bias and jg == ntg - 1))
                                jg += 1
                        # rank-1 bias: psg += binv_g (x) b_row
                        co = C if lay < 2 else COUT
                        if has_bias:
                            nc.tensor.matmul(
                                out=psg[:, :co],
                                lhsT=binv_sb[:, g * 128:(g + 1) * 128],
                                rhs=br_sb[lay][:], start=False, stop=True)
                        if lay < 2:
                            h = wpool.tile([128, 128], f32, name="h")
                            nc.scalar.activation(
                                out=h[:], in_=psg[:],
                                func=mybir.ActivationFunctionType.Relu,
                                scale=dinv_sb[:, g:g + 1])
                            pst = ps_t.tile([128, 128], f32, name="pst")
                            nc.tensor.transpose(out=pst[:], in_=h[:],
                                                identity=ident[:])
                            ht = wpool.tile([128, 128], f32, name="ht")
                            nc.vector.tensor_copy(out=ht[:], in_=pst[:])
                            co2 = C if lay == 0 else COUT
                            psz = ps_z.tile([128, C], f32, name="psz2")
                            nc.tensor.matmul(out=psz[:, :co2], lhsT=ht[:],
                                             rhs=w_sb[lay + 1][:],
                                             start=True, stop=True)
                            zt = wpool.tile([128, C], bf16, name="zt2")
                            nc.vector.tensor_scalar_mul(
                                out=zt[:, :co2], in0=psz[:, :co2],
                                scalar1=dinv_sb[:, g:g + 1])
                            nc.sync.dma_start(
                                out=zs[lay + 1][g * 128:(g + 1) * 128, :co2],
                                in_=zt[:, :co2])
                        else:
                            nc.vector.tensor_scalar_mul(
                                out=smbuf[:, g, :], in0=psg[:, :COUT],
                                scalar1=dinv_sb[:, g:g + 1])
                if lay < 2:
                    nc.gpsimd.collective_compute(
                        "AllGather", mybir.AluOpType.bypass,
                        replica_groups=[list(range(NCORES))],
                        ins=[zs[lay + 1][:, :]], outs=[zf[lay + 1][:, :]])

            # ---- batched log_softmax over all 98 groups (in-place) ----
            nc.vector.tensor_reduce(
                out=smx[:], in_=smbuf[:], axis=mybir.AxisListType.X,
                op=mybir.AluOpType.max)
            nc.vector.tensor_tensor(
                out=smbuf[:], in0=smbuf[:],
                in1=smx[:].to_broadcast([128, NGRP, COUT]),
                op=mybir.AluOpType.subtract)
            ex = cpool.tile([128, NGRP, COUT], f32, name="exb")
            nc.scalar.activation(
                out=ex[:], in_=smbuf[:],
                func=mybir.ActivationFunctionType.Exp)
            nc.vector.tensor_reduce(
                out=sls[:], in_=ex[:], axis=mybir.AxisListType.X,
                op=mybir.AluOpType.add)
            nc.scalar.activation(
                out=sls[:], in_=sls[:],
                func=mybir.ActivationFunctionType.Ln)
            nc.vector.tensor_tensor(
                out=smbuf[:], in0=smbuf[:],
                in1=sls[:].to_broadcast([128, NGRP, COUT]),
                op=mybir.AluOpType.subtract)
            for g in range(NGRP):
                nc.sync.dma_start(
                    out=out_d[g * 128:(g + 1) * 128, :], in_=smbuf[:, g, :])

    nc.compile()
    return nc


LAST_RES = None


def kernel(x, edge_index, W1, b1, W2, b2, W3, b3):
    import os
    from concourse.bass_utils import run_bass_kernel_spmd

    in_maps, meta, perm = _preprocess(
        x, edge_index, W1, b1, W2, b2, W3, b3)
    nc = _build(meta)
    kw = {}
    if os.environ.get("KERNEL_TRACE", "0") == "1":
        kw["trace"] = True
        if os.environ.get("KERNEL_TMPDIR"):
            kw["tmpdir"] = os.environ["KERNEL_TMPDIR"]
    res = run_bass_kernel_spmd(nc, in_maps, core_ids=list(range(NCORES)), **kw)
    global LAST_RES
    LAST_RES = res
    blocks = [res.results[k]["out"][:NBLK] for k in range(NCORES)]
    outp = np.concatenate(blocks, axis=0)
    out = np.empty((N, COUT), np.float32)
    out[perm] = outp
    return out
